# revision 11
# baseline (speedup 1.0000x reference)
"""Trainium2 Bass kernel for nn_DRO_TOPK (margin-loss top-k + masked sim stats).

Strategy (8 NeuronCores, data-parallel over rows, symmetry-halved):
  - sim = X @ X.T is symmetric: every unordered pair {i, j} is covered once
    by the half-circle band d = (j - i) mod 4096 in [1, 2048]. Each core
    computes, for its 512 rows, a [128, 2176]-wide rectangle per row-tile
    (cols [a, a+2176) in core-local rotated coordinates, a = t*128) that
    covers each row's band plus <=128 junk cells/row (diag + mirror
    duplicates), which the host filters out by index.
  - Per-core inputs are column-ROTATED by c*512 so the band always sits at
    local cols [0, 2560) -> one uniform SPMD program; only 5 of 8 MB of
    X^T per core is ever touched.
  - On chip: w[i,j] = (notsame - 0.5) * sim in {-s/2, +s/2}; pair_loss =
    relu(MARGIN + 2*w), monotone in w. Device emits per-row top-8 of w
    (max8 + max_index) and Sign-bracketed zero-loss counts on the Scalar
    engine. Matmuls run as float32r (1 cyc/row at N>=256).
  - Host: drops junk by index (d outside [1,2048]), recomputes surviving
    candidate sims exactly in f64, takes top-10 unique pairs (x2 = the
    reference's top-20), and computes mean_pos/mean_neg/counts exactly in
    f64. Guards (top-8 sufficiency, zero-count bracket) trigger a full
    numpy fallback if the fast path cannot be proven exact.
"""

import os
import sys

import numpy as np

for _p in ('/opt/trn_rl_repo', '/root/.axon_site/_ro/trn_rl_repo'):
    if os.path.isdir(_p) and _p not in sys.path:
        sys.path.insert(0, _p)

N, D, NCORES = 4096, 512, 8
R = N // NCORES            # 512 rows per core
NT = R // 128              # 4 row-tiles per core
HB = N // 2                # 2048 half-circle band width
W_RECT = HB + 128          # 2176 rect width per row-tile
XCOLS = 3 * 128 + W_RECT   # 2560 cols of rotated X^T each core touches
KK = D // 128              # 4 contraction sub-tiles
MARGIN, BETA, TOPK = 0.5, 0.0, 20
ZTHR = -MARGIN / 2.0       # w <= ZTHR  <=>  pair_loss == 0
DELTA = 1e-3               # zero-count bracket width

_prog_cache = {}


def _round_f32r(a):
    """Round f32 array to float32r (RN to 11 mantissa bits), so the on-device
    f32r matmul consumes exactly these values via a non-casting DMA."""
    bits = a.astype(np.float32).view(np.uint32)
    rnd = ((bits.astype(np.uint64) + 0x800) >> 12 << 12).astype(np.uint32)
    return rnd.view(np.float32)


def _build_program():
    import concourse.bacc as bacc
    import concourse.mybir as mybir
    from concourse.tile import TileContext

    f32 = mybir.dt.float32
    f16 = mybir.dt.float16
    u32 = mybir.dt.uint32
    f32r = mybir.dt.float32r
    Alu = mybir.AluOpType
    Act = mybir.ActivationFunctionType

    nc = bacc.Bacc('TRN2', target_bir_lowering=False, debug=False)
    xtr_d = nc.dram_tensor('xtr', [KK, 128, XCOLS], f32r, kind='ExternalInput')
    tgr_d = nc.dram_tensor('tgr', [XCOLS], f16, kind='ExternalInput')
    tgf_d = nc.dram_tensor('tgf', [128, NT], f32, kind='ExternalInput')
    # fused output, device-native layout: per partition p:
    # [cand(t,j): 32 | candi(t,j) u32-bits: 32 | zlo(t): 4 | zhi(t): 4]
    outp_d = nc.dram_tensor('outp', [128, 72], f32, kind='ExternalOutput')

    with TileContext(nc) as tc:
        with (
            tc.tile_pool(name='xts', bufs=1) as xts_pool,
            tc.tile_pool(name='tb', bufs=1) as tb_pool,
            tc.tile_pool(name='w', bufs=2) as w_pool,
            tc.tile_pool(name='mb', bufs=2) as mb_pool,
            tc.tile_pool(name='zs', bufs=1) as zs_pool,
            tc.tile_pool(name='small', bufs=1) as small_pool,
            tc.tile_pool(name='psb', bufs=3, space='PSUM') as psb_pool,
            tc.tile_pool(name='pst', bufs=2, space='PSUM') as pst_pool,
        ):
            # Rotated X^T in SBUF: 4 partition-tiles of [128, 2560] f32r,
            # each a single contiguous-per-partition DMA (fat descriptors).
            xts = [xts_pool.tile([128, XCOLS], f32r, tag=f'xt{kk}',
                                 name=f'xts{kk}') for kk in range(KK)]
            # two pieces per tile: the first 1MB unblocks the PE early
            for kk in range(KK):
                nc.sync.dma_start(xts[kk][:, 0:1024], xtr_d[kk, :, 0:1024])
            for kk in range(KK):
                nc.sync.dma_start(xts[kk][:, 1024:XCOLS],
                                  xtr_d[kk, :, 1024:XCOLS])

            # Rotated targets (f16) broadcast to all 128 partitions.
            tb = tb_pool.tile([128, XCOLS], f16)
            nc.sync.dma_start(tb[:, :], tgr_d[:].unsqueeze(0).partition_broadcast(128))
            # Per-partition row targets (f32): tr[p, t] = target[t*128 + p].
            tr = small_pool.tile([128, NT], f32, tag='tr')
            nc.sync.dma_start(tr[:, :], tgf_d[:, :])

            outt = small_pool.tile([128, 72], f32, tag='outt')
            bias_hi = small_pool.tile([128, 1], f32, tag='bias_hi')
            nc.vector.memset(bias_hi[:, :], -(ZTHR + DELTA))
            bias_lo = small_pool.tile([128, 1], f32, tag='bias_lo')
            nc.vector.memset(bias_lo[:, :], -(ZTHR - DELTA))

            for t in range(NT):
                a = t * 128
                # (notsame - 0.5) in fp16 for this row-tile's rect.
                mb = mb_pool.tile([128, W_RECT], f16)
                nc.vector.tensor_scalar(mb[:, :], tb[:, a:a + W_RECT],
                                        tr[:, t:t + 1], 0.5,
                                        Alu.not_equal, Alu.subtract)
                w = w_pool.tile([128, W_RECT], f32)
                # band pieces: 2x [128,1024] (2 PSUM banks) + 1x [128,128]
                for piece in range(2):
                    ps = psb_pool.tile([128, 1024], f32, name=f'psb{t}_{piece}',
                                       tag='psb')
                    for h in range(2):
                        o = a + piece * 1024 + h * 512
                        for kk in range(KK):
                            nc.tensor.matmul(ps[:, h * 512:(h + 1) * 512],
                                             xts[kk][:, a:a + 128],
                                             xts[kk][:, o:o + 512],
                                             start=(kk == 0), stop=(kk == KK - 1))
                    nc.vector.tensor_tensor(
                        w[:, piece * 1024:(piece + 1) * 1024],
                        mb[:, piece * 1024:(piece + 1) * 1024], ps[:, :],
                        op=Alu.mult)
                pt = pst_pool.tile([128, 128], f32, tag='pst')
                o = a + 2048
                for kk in range(KK):
                    nc.tensor.matmul(pt[:, :], xts[kk][:, a:a + 128],
                                     xts[kk][:, o:o + 128],
                                     start=(kk == 0), stop=(kk == KK - 1))
                nc.vector.tensor_tensor(w[:, 2048:2176], mb[:, 2048:2176],
                                        pt[:, :], op=Alu.mult)
                # Per-row top-8 candidates of w (+ their rect indices).
                nc.vector.max(outt[:, t * 8:(t + 1) * 8], w[:, :])
                nc.vector.max_index(
                    outt[:, 32 + t * 8:32 + (t + 1) * 8].bitcast(u32),
                    outt[:, t * 8:(t + 1) * 8], w[:, :])
                # Sign-bracketed zero-loss counts on ACT.
                z1 = zs_pool.tile([128, W_RECT], f32, tag='z1')
                nc.scalar.activation(z1[:, :], w[:, :], Act.Sign,
                                     bias=bias_hi[:, :],
                                     accum_out=outt[:, 68 + t:69 + t])
                z2 = zs_pool.tile([128, W_RECT], f32, tag='z2')
                nc.scalar.activation(z2[:, :], w[:, :], Act.Sign,
                                     bias=bias_lo[:, :],
                                     accum_out=outt[:, 64 + t:65 + t])

            # split across partition ranges so 8 DMA queues work in parallel
            for q in range(8):
                nc.sync.dma_start(outp_d[q * 16:(q + 1) * 16, :],
                                  outt[q * 16:(q + 1) * 16, :])

    nc.compile()
    return nc


def _numpy_fallback(x, t):
    """Faithful f32 numpy recompute of the full reference (safety net)."""
    sim = x @ x.T
    same = t[:, None] == t[None, :]
    eye = np.eye(N, dtype=bool)
    pos = same & ~eye
    neg = ~same
    pos_l = np.maximum(MARGIN + BETA - sim, 0.0).astype(np.float32)
    neg_l = np.maximum(MARGIN + sim - BETA, 0.0).astype(np.float32)
    valid = pos | neg
    pair = np.where(pos, pos_l, neg_l)
    zeros = int((valid & (pair == 0.0)).sum())
    masked = np.where(valid, pair, -np.inf).ravel()
    top = np.sort(masked)[-TOPK:]
    loss = np.float32(top.astype(np.float64).mean())
    mean_pos = np.float32(sim[pos].astype(np.float64).sum() / pos.sum())
    mean_neg = np.float32(sim[neg].astype(np.float64).sum() / neg.sum())
    return loss, np.int32(zeros), mean_pos, mean_neg


def kernel(**inputs):
    from concourse.bass_utils import run_bass_kernel_spmd

    x = np.ascontiguousarray(inputs['inputs'].astype(np.float32, copy=False))
    t = np.asarray(inputs['targets'])
    t_i = t.astype(np.int64)
    t16 = t.astype(np.float16)
    t32 = t.astype(np.float32)

    if 'nc' not in _prog_cache:
        _prog_cache['nc'] = _build_program()
    nc = _prog_cache['nc']

    xt = _round_f32r(np.ascontiguousarray(x.T))          # [D, N] f32r values
    xt2 = np.concatenate([xt, xt[:, :XCOLS - N]], axis=1)   # wrap for rotation
    t16w = np.concatenate([t16, t16[:XCOLS - N]])
    in_maps = []
    for c in range(NCORES):
        sh = c * R
        in_maps.append({
            'xtr': np.ascontiguousarray(
                xt2[:, sh:sh + XCOLS].reshape(KK, 128, XCOLS)),
            'tgr': np.ascontiguousarray(t16w[sh:sh + XCOLS]),
            'tgf': np.ascontiguousarray(
                t32[sh:sh + R].reshape(NT, 128).T),
        })

    res = run_bass_kernel_spmd(nc, in_maps, core_ids=list(range(NCORES)))

    cands, candis, zlos, zhis = [], [], [], []
    for r in res.results:
        o = r['outp']                                   # [128, 72]
        cands.append(o[:, 0:32].reshape(128, NT, 8).transpose(1, 0, 2)
                     .reshape(R, 8))
        candis.append(o[:, 32:64].view(np.uint32).reshape(128, NT, 8)
                      .transpose(1, 0, 2).reshape(R, 8))
        zlos.append(o[:, 64:68].T.reshape(R))
        zhis.append(o[:, 68:72].T.reshape(R))
    cand = np.concatenate(cands, axis=0)                # [N, 8]
    candi = np.concatenate(candis, axis=0).astype(np.int64)
    zsum_lo = np.concatenate(zlos)                      # [N]
    zsum_hi = np.concatenate(zhis)

    # ---- index bookkeeping: row r of cand is global row (r), candidate x
    # refers to core-local col a + x, global col (c*512 + a + x) mod N,
    # where a = (r % 512) // 128 * 128. d = x - p, p = r % 128. ----
    rows = np.arange(N)
    p = rows % 128
    d = candi - p[:, None]                       # pair distance, [N, 8]
    gcol = (rows[:, None] - p[:, None] + candi) % N
    keep = (d >= 1) & ((d <= HB - 1) | ((d == HB) & (rows[:, None] < gcol)))

    # ---- exact f64 values for kept candidates ----
    x64 = x.astype(np.float64)
    ri, ci = np.nonzero(keep)
    gi = rows[ri]
    gj = gcol[ri, ci]
    s_exact = np.einsum('nd,nd->n', x64[gi], x64[gj])
    sign = np.where(t_i[gi] == t_i[gj], -1.0, 1.0)
    w_exact = 0.5 * sign * s_exact

    # ---- top-10 unique pairs (x2 = reference top-20) ----
    order = np.argsort(w_exact)[::-1]
    top10 = w_exact[order[:TOPK // 2]]
    t10_dev = np.sort(cand[keep].ravel())[-(TOPK // 2)]
    sufficiency_ok = (len(w_exact) >= TOPK // 2 and
                      bool((cand[:, 7] <= t10_dev).all()))

    # ---- zero count guards (expect exactly the diag junk cell per row) ----
    c_lo = (W_RECT - zsum_lo) / 2.0
    c_hi = (W_RECT - zsum_hi) / 2.0
    zeros_ok = (np.all(c_lo == 1.0) and np.all(c_hi == 1.0))
    if not (sufficiency_ok and zeros_ok):
        return _numpy_fallback(x, t_i)
    num_zeros = 0

    top20 = np.repeat(top10, 2)
    loss = np.float32(np.maximum(MARGIN + 2.0 * top20, 0.0).mean())

    # ---- exact f64 stats on host ----
    G = np.zeros((int(t_i.max()) + 1, D), dtype=np.float64)
    np.add.at(G, t_i, x64)
    cls_sq = float((G * G).sum())
    diag_sq = float((x64 * x64).sum())
    cnt = np.bincount(t_i)
    pos_cnt = int((cnt.astype(np.int64) * (cnt - 1)).sum())
    neg_cnt = N * N - int((cnt.astype(np.int64) ** 2).sum())
    tot = x64.sum(axis=0)
    total_sq = float(tot @ tot)
    mean_pos = np.float32((cls_sq - diag_sq) / pos_cnt)
    mean_neg = np.float32((total_sq - cls_sq) / neg_cnt)

    return loss, np.int32(num_zeros), mean_pos, mean_neg


# revision 12
# speedup vs baseline: 1.0865x; 1.0865x over previous
"""Trainium2 Bass kernel for nn_DRO_TOPK (margin-loss top-k + masked sim stats).

Strategy (8 NeuronCores, data-parallel over rows, symmetry-halved):
  - sim = X @ X.T is symmetric: every unordered pair {i, j} is covered once
    by the half-circle band d = (j - i) mod 4096 in [1, 2048]. Each core
    computes, for its 512 rows, a [128, 2176]-wide rectangle per row-tile
    (cols [a, a+2176) in core-local rotated coordinates, a = t*128) that
    covers each row's band plus <=128 junk cells/row (diag + mirror
    duplicates), which the host filters out by index.
  - Per-core inputs are column-ROTATED by c*512 so the band always sits at
    local cols [0, 2560) -> one uniform SPMD program; only 5 of 8 MB of
    X^T per core is ever touched.
  - On chip: w[i,j] = (notsame - 0.5) * sim in {-s/2, +s/2}; pair_loss =
    relu(MARGIN + 2*w), monotone in w. Device emits per-row top-8 of w
    (max8 + max_index) and Sign-bracketed zero-loss counts on the Scalar
    engine. Matmuls run as float32r (1 cyc/row at N>=256).
  - Host: drops junk by index (d outside [1,2048]), recomputes surviving
    candidate sims exactly in f64, takes top-10 unique pairs (x2 = the
    reference's top-20), and computes mean_pos/mean_neg/counts exactly in
    f64. Guards (top-8 sufficiency, zero-count bracket) trigger a full
    numpy fallback if the fast path cannot be proven exact.
"""

import os
import sys

import numpy as np

for _p in ('/opt/trn_rl_repo', '/root/.axon_site/_ro/trn_rl_repo'):
    if os.path.isdir(_p) and _p not in sys.path:
        sys.path.insert(0, _p)

N, D, NCORES = 4096, 512, 8
R = N // NCORES            # 512 rows per core
NT = R // 128              # 4 row-tiles per core
HB = N // 2                # 2048 half-circle band width
W_RECT = HB + 128          # 2176 rect width per row-tile
XCOLS = 3 * 128 + W_RECT   # 2560 cols of rotated X^T each core touches
KK = D // 128              # 4 contraction sub-tiles
MARGIN, BETA, TOPK = 0.5, 0.0, 20
ZTHR = -MARGIN / 2.0       # w <= ZTHR  <=>  pair_loss == 0
DELTA = 1e-3               # zero-count bracket width

_prog_cache = {}


def _round_f32r(a):
    """Round f32 array to float32r (RN to 11 mantissa bits), so the on-device
    f32r matmul consumes exactly these values via a non-casting DMA."""
    bits = a.astype(np.float32).view(np.uint32)
    rnd = ((bits.astype(np.uint64) + 0x800) >> 12 << 12).astype(np.uint32)
    return rnd.view(np.float32)


def _build_program():
    import concourse.bacc as bacc
    import concourse.mybir as mybir
    from concourse.tile import TileContext

    f32 = mybir.dt.float32
    f16 = mybir.dt.float16
    u32 = mybir.dt.uint32
    f32r = mybir.dt.float32r
    Alu = mybir.AluOpType
    Act = mybir.ActivationFunctionType

    nc = bacc.Bacc('TRN2', target_bir_lowering=False, debug=False)
    xtr_d = nc.dram_tensor('xtr', [KK, 128, XCOLS], f32r, kind='ExternalInput')
    tgr_d = nc.dram_tensor('tgr', [XCOLS], f16, kind='ExternalInput')
    tgf_d = nc.dram_tensor('tgf', [128, NT], f32, kind='ExternalInput')
    # fused output, device-native layout: per partition p:
    # [cand(t,j): 32 | candi(t,j) u32-bits: 32 | zlo(t): 4 | zhi(t): 4]
    outp_d = nc.dram_tensor('outp', [128, 72], f32, kind='ExternalOutput')

    with TileContext(nc) as tc:
        with (
            tc.tile_pool(name='xts', bufs=1) as xts_pool,
            tc.tile_pool(name='tb', bufs=1) as tb_pool,
            tc.tile_pool(name='w', bufs=2) as w_pool,
            tc.tile_pool(name='mb', bufs=2) as mb_pool,
            tc.tile_pool(name='zs', bufs=1) as zs_pool,
            tc.tile_pool(name='small', bufs=1) as small_pool,
            tc.tile_pool(name='psb', bufs=3, space='PSUM') as psb_pool,
            tc.tile_pool(name='pst', bufs=2, space='PSUM') as pst_pool,
        ):
            # Rotated X^T in SBUF: 4 partition-tiles of [128, 2560] f32r,
            # each a single contiguous-per-partition DMA (fat descriptors).
            xts = [xts_pool.tile([128, XCOLS], f32r, tag=f'xt{kk}',
                                 name=f'xts{kk}') for kk in range(KK)]
            for kk in range(KK):
                nc.sync.dma_start(xts[kk][:, :], xtr_d[kk, :, :])

            # Rotated targets (f16) broadcast to all 128 partitions.
            tb = tb_pool.tile([128, XCOLS], f16)
            nc.sync.dma_start(tb[:, :], tgr_d[:].unsqueeze(0).partition_broadcast(128))
            # Per-partition row targets (f32): tr[p, t] = target[t*128 + p].
            tr = small_pool.tile([128, NT], f32, tag='tr')
            nc.sync.dma_start(tr[:, :], tgf_d[:, :])

            outt = small_pool.tile([128, 72], f32, tag='outt')
            bias_hi = small_pool.tile([128, 1], f32, tag='bias_hi')
            nc.vector.memset(bias_hi[:, :], -(ZTHR + DELTA))
            bias_lo = small_pool.tile([128, 1], f32, tag='bias_lo')
            nc.vector.memset(bias_lo[:, :], -(ZTHR - DELTA))

            for t in range(NT):
                a = t * 128
                # (notsame - 0.5) in fp16 for this row-tile's rect.
                mb = mb_pool.tile([128, W_RECT], f16)
                nc.vector.tensor_scalar(mb[:, :], tb[:, a:a + W_RECT],
                                        tr[:, t:t + 1], 0.5,
                                        Alu.not_equal, Alu.subtract)
                w = w_pool.tile([128, W_RECT], f32)
                # band pieces: 2x [128,1024] (2 PSUM banks) + 1x [128,128]
                for piece in range(2):
                    ps = psb_pool.tile([128, 1024], f32, name=f'psb{t}_{piece}',
                                       tag='psb')
                    for h in range(2):
                        o = a + piece * 1024 + h * 512
                        for kk in range(KK):
                            nc.tensor.matmul(ps[:, h * 512:(h + 1) * 512],
                                             xts[kk][:, a:a + 128],
                                             xts[kk][:, o:o + 512],
                                             start=(kk == 0), stop=(kk == KK - 1))
                    nc.vector.tensor_tensor(
                        w[:, piece * 1024:(piece + 1) * 1024],
                        mb[:, piece * 1024:(piece + 1) * 1024], ps[:, :],
                        op=Alu.mult)
                pt = pst_pool.tile([128, 128], f32, tag='pst')
                o = a + 2048
                for kk in range(KK):
                    nc.tensor.matmul(pt[:, :], xts[kk][:, a:a + 128],
                                     xts[kk][:, o:o + 128],
                                     start=(kk == 0), stop=(kk == KK - 1))
                nc.vector.tensor_tensor(w[:, 2048:2176], mb[:, 2048:2176],
                                        pt[:, :], op=Alu.mult)
                # Per-row top-8 candidates of w (+ their rect indices).
                nc.vector.max(outt[:, t * 8:(t + 1) * 8], w[:, :])
                nc.vector.max_index(
                    outt[:, 32 + t * 8:32 + (t + 1) * 8].bitcast(u32),
                    outt[:, t * 8:(t + 1) * 8], w[:, :])
                # Sign-bracketed zero-loss counts on ACT.
                z1 = zs_pool.tile([128, W_RECT], f32, tag='z1')
                nc.scalar.activation(z1[:, :], w[:, :], Act.Sign,
                                     bias=bias_hi[:, :],
                                     accum_out=outt[:, 68 + t:69 + t])
                z2 = zs_pool.tile([128, W_RECT], f32, tag='z2')
                nc.scalar.activation(z2[:, :], w[:, :], Act.Sign,
                                     bias=bias_lo[:, :],
                                     accum_out=outt[:, 64 + t:65 + t])

            nc.sync.dma_start(outp_d[:, :], outt[:, :])

    nc.compile()
    return nc


def _numpy_fallback(x, t):
    """Faithful f32 numpy recompute of the full reference (safety net)."""
    sim = x @ x.T
    same = t[:, None] == t[None, :]
    eye = np.eye(N, dtype=bool)
    pos = same & ~eye
    neg = ~same
    pos_l = np.maximum(MARGIN + BETA - sim, 0.0).astype(np.float32)
    neg_l = np.maximum(MARGIN + sim - BETA, 0.0).astype(np.float32)
    valid = pos | neg
    pair = np.where(pos, pos_l, neg_l)
    zeros = int((valid & (pair == 0.0)).sum())
    masked = np.where(valid, pair, -np.inf).ravel()
    top = np.sort(masked)[-TOPK:]
    loss = np.float32(top.astype(np.float64).mean())
    mean_pos = np.float32(sim[pos].astype(np.float64).sum() / pos.sum())
    mean_neg = np.float32(sim[neg].astype(np.float64).sum() / neg.sum())
    return loss, np.int32(zeros), mean_pos, mean_neg


def kernel(**inputs):
    from concourse.bass_utils import run_bass_kernel_spmd

    x = np.ascontiguousarray(inputs['inputs'].astype(np.float32, copy=False))
    t = np.asarray(inputs['targets'])
    t_i = t.astype(np.int64)
    t16 = t.astype(np.float16)
    t32 = t.astype(np.float32)

    if 'nc' not in _prog_cache:
        _prog_cache['nc'] = _build_program()
    nc = _prog_cache['nc']

    xt = _round_f32r(np.ascontiguousarray(x.T))          # [D, N] f32r values
    xt2 = np.concatenate([xt, xt[:, :XCOLS - N]], axis=1)   # wrap for rotation
    t16w = np.concatenate([t16, t16[:XCOLS - N]])
    in_maps = []
    for c in range(NCORES):
        sh = c * R
        in_maps.append({
            'xtr': np.ascontiguousarray(
                xt2[:, sh:sh + XCOLS].reshape(KK, 128, XCOLS)),
            'tgr': np.ascontiguousarray(t16w[sh:sh + XCOLS]),
            'tgf': np.ascontiguousarray(
                t32[sh:sh + R].reshape(NT, 128).T),
        })

    res = run_bass_kernel_spmd(nc, in_maps, core_ids=list(range(NCORES)))

    cands, candis, zlos, zhis = [], [], [], []
    for r in res.results:
        o = r['outp']                                   # [128, 72]
        cands.append(o[:, 0:32].reshape(128, NT, 8).transpose(1, 0, 2)
                     .reshape(R, 8))
        candis.append(o[:, 32:64].view(np.uint32).reshape(128, NT, 8)
                      .transpose(1, 0, 2).reshape(R, 8))
        zlos.append(o[:, 64:68].T.reshape(R))
        zhis.append(o[:, 68:72].T.reshape(R))
    cand = np.concatenate(cands, axis=0)                # [N, 8]
    candi = np.concatenate(candis, axis=0).astype(np.int64)
    zsum_lo = np.concatenate(zlos)                      # [N]
    zsum_hi = np.concatenate(zhis)

    # ---- index bookkeeping: row r of cand is global row (r), candidate x
    # refers to core-local col a + x, global col (c*512 + a + x) mod N,
    # where a = (r % 512) // 128 * 128. d = x - p, p = r % 128. ----
    rows = np.arange(N)
    p = rows % 128
    d = candi - p[:, None]                       # pair distance, [N, 8]
    gcol = (rows[:, None] - p[:, None] + candi) % N
    keep = (d >= 1) & ((d <= HB - 1) | ((d == HB) & (rows[:, None] < gcol)))

    # ---- exact f64 values for kept candidates ----
    x64 = x.astype(np.float64)
    ri, ci = np.nonzero(keep)
    gi = rows[ri]
    gj = gcol[ri, ci]
    s_exact = np.einsum('nd,nd->n', x64[gi], x64[gj])
    sign = np.where(t_i[gi] == t_i[gj], -1.0, 1.0)
    w_exact = 0.5 * sign * s_exact

    # ---- top-10 unique pairs (x2 = reference top-20) ----
    order = np.argsort(w_exact)[::-1]
    top10 = w_exact[order[:TOPK // 2]]
    t10_dev = np.sort(cand[keep].ravel())[-(TOPK // 2)]
    sufficiency_ok = (len(w_exact) >= TOPK // 2 and
                      bool((cand[:, 7] <= t10_dev).all()))

    # ---- zero count guards (expect exactly the diag junk cell per row) ----
    c_lo = (W_RECT - zsum_lo) / 2.0
    c_hi = (W_RECT - zsum_hi) / 2.0
    zeros_ok = (np.all(c_lo == 1.0) and np.all(c_hi == 1.0))
    if not (sufficiency_ok and zeros_ok):
        return _numpy_fallback(x, t_i)
    num_zeros = 0

    top20 = np.repeat(top10, 2)
    loss = np.float32(np.maximum(MARGIN + 2.0 * top20, 0.0).mean())

    # ---- exact f64 stats on host ----
    G = np.zeros((int(t_i.max()) + 1, D), dtype=np.float64)
    np.add.at(G, t_i, x64)
    cls_sq = float((G * G).sum())
    diag_sq = float((x64 * x64).sum())
    cnt = np.bincount(t_i)
    pos_cnt = int((cnt.astype(np.int64) * (cnt - 1)).sum())
    neg_cnt = N * N - int((cnt.astype(np.int64) ** 2).sum())
    tot = x64.sum(axis=0)
    total_sq = float(tot @ tot)
    mean_pos = np.float32((cls_sq - diag_sq) / pos_cnt)
    mean_neg = np.float32((total_sq - cls_sq) / neg_cnt)

    return loss, np.int32(num_zeros), mean_pos, mean_neg


# revision 14
# speedup vs baseline: 1.1558x; 1.0637x over previous
"""Trainium2 Bass kernel for nn_DRO_TOPK (margin-loss top-k + masked sim stats).

Strategy (8 NeuronCores, data-parallel over rows, symmetry-halved):
  - sim = X @ X.T is symmetric: every unordered pair {i, j} is covered once
    by the half-circle band d = (j - i) mod 4096 in [1, 2048]. Each core
    computes, for its 512 rows, a [128, 2176]-wide rectangle per row-tile
    (cols [a, a+2176) in core-local rotated coordinates, a = t*128) that
    covers each row's band plus <=128 junk cells/row (diag + mirror
    duplicates), which the host filters out by index.
  - Per-core inputs are column-ROTATED by c*512 so the band always sits at
    local cols [0, 2560) -> one uniform SPMD program; only 5 of 8 MB of
    X^T per core is ever touched.
  - On chip: w[i,j] = (notsame - 0.5) * sim in {-s/2, +s/2}; pair_loss =
    relu(MARGIN + 2*w), monotone in w. Device emits per-row top-8 of w
    (max8 + max_index) and Sign-bracketed zero-loss counts on the Scalar
    engine. Matmuls run as float32r (1 cyc/row at N>=256).
  - Host: drops junk by index (d outside [1,2048]), recomputes surviving
    candidate sims exactly in f64, takes top-10 unique pairs (x2 = the
    reference's top-20), and computes mean_pos/mean_neg/counts exactly in
    f64. Guards (top-8 sufficiency, zero-count bracket) trigger a full
    numpy fallback if the fast path cannot be proven exact.
"""

import os
import sys

import numpy as np

for _p in ('/opt/trn_rl_repo', '/root/.axon_site/_ro/trn_rl_repo'):
    if os.path.isdir(_p) and _p not in sys.path:
        sys.path.insert(0, _p)

N, D, NCORES = 4096, 512, 8
R = N // NCORES            # 512 rows per core
NT = R // 128              # 4 row-tiles per core
HB = N // 2                # 2048 half-circle band width
W_RECT = HB + 128          # 2176 rect width per row-tile
XCOLS = 3 * 128 + W_RECT   # 2560 cols of rotated X^T each core touches
KK = D // 128              # 4 contraction sub-tiles
MARGIN, BETA, TOPK = 0.5, 0.0, 20
ZTHR = -MARGIN / 2.0       # w <= ZTHR  <=>  pair_loss == 0
DELTA = 1e-3               # zero-count bracket width

_prog_cache = {}


def _round_f32r(a):
    """Round f32 array to float32r (RN to 11 mantissa bits), so the on-device
    f32r matmul consumes exactly these values via a non-casting DMA."""
    bits = a.astype(np.float32).view(np.uint32)
    rnd = ((bits.astype(np.uint64) + 0x800) >> 12 << 12).astype(np.uint32)
    return rnd.view(np.float32)


def _build_program():
    import concourse.bacc as bacc
    import concourse.mybir as mybir
    from concourse.tile import TileContext

    f32 = mybir.dt.float32
    f16 = mybir.dt.float16
    u32 = mybir.dt.uint32
    f32r = mybir.dt.float32r
    Alu = mybir.AluOpType
    Act = mybir.ActivationFunctionType

    nc = bacc.Bacc('TRN2', target_bir_lowering=False, debug=False)
    xtr_d = nc.dram_tensor('xtr', [KK, 128, XCOLS], f32r, kind='ExternalInput')
    tgr_d = nc.dram_tensor('tgr', [XCOLS], f16, kind='ExternalInput')
    tgf_d = nc.dram_tensor('tgf', [128, NT], f32, kind='ExternalInput')
    jmask_d = nc.dram_tensor('jmask', [128, W_RECT], f16, kind='ExternalInput')
    # fused output, device-native layout: per partition p:
    # [cand(t,j): 32 | zlo(t): 4 | zhi(t): 4]
    outp_d = nc.dram_tensor('outp', [128, 40], f32, kind='ExternalOutput')

    with TileContext(nc) as tc:
        with (
            tc.tile_pool(name='xts', bufs=1) as xts_pool,
            tc.tile_pool(name='tb', bufs=1) as tb_pool,
            tc.tile_pool(name='w', bufs=2) as w_pool,
            tc.tile_pool(name='mb', bufs=2) as mb_pool,
            tc.tile_pool(name='zs', bufs=1) as zs_pool,
            tc.tile_pool(name='small', bufs=1) as small_pool,
            tc.tile_pool(name='psb', bufs=3, space='PSUM') as psb_pool,
            tc.tile_pool(name='pst', bufs=2, space='PSUM') as pst_pool,
        ):
            # Rotated X^T in SBUF: 4 partition-tiles of [128, 2560] f32r,
            # each a single contiguous-per-partition DMA (fat descriptors).
            xts = [xts_pool.tile([128, XCOLS], f32r, tag=f'xt{kk}',
                                 name=f'xts{kk}') for kk in range(KK)]
            for kk in range(KK):
                nc.sync.dma_start(xts[kk][:, :], xtr_d[kk, :, :])

            # Rotated targets (f16) broadcast to all 128 partitions.
            tb = tb_pool.tile([128, XCOLS], f16)
            nc.sync.dma_start(tb[:, :], tgr_d[:].unsqueeze(0).partition_broadcast(128))
            # Per-partition row targets (f32): tr[p, t] = target[t*128 + p].
            tr = small_pool.tile([128, NT], f32, tag='tr')
            nc.sync.dma_start(tr[:, :], tgf_d[:, :])
            # band mask J[p, x] = 1 iff 1 <= x - p <= 2047 (junk cells -> 0)
            jm = small_pool.tile([128, W_RECT], f16, tag='jm')
            nc.sync.dma_start(jm[:, :], jmask_d[:, :])

            outt = small_pool.tile([128, 40], f32, tag='outt')
            bias_hi = small_pool.tile([128, 1], f32, tag='bias_hi')
            nc.vector.memset(bias_hi[:, :], -(ZTHR + DELTA))
            bias_lo = small_pool.tile([128, 1], f32, tag='bias_lo')
            nc.vector.memset(bias_lo[:, :], -(ZTHR - DELTA))

            for t in range(NT):
                a = t * 128
                # (notsame - 0.5) in fp16, then band-masked by J.
                mb0 = mb_pool.tile([128, W_RECT], f16, tag='mb0')
                nc.vector.tensor_scalar(mb0[:, :], tb[:, a:a + W_RECT],
                                        tr[:, t:t + 1], 0.5,
                                        Alu.not_equal, Alu.subtract)
                mb = mb_pool.tile([128, W_RECT], f16, tag='mb')
                nc.vector.tensor_tensor(mb[:, :], mb0[:, :], jm[:, :],
                                        op=Alu.mult)
                w = w_pool.tile([128, W_RECT], f32)
                # band pieces: 2x [128,1024] (2 PSUM banks) + 1x [128,128]
                for piece in range(2):
                    ps = psb_pool.tile([128, 1024], f32, name=f'psb{t}_{piece}',
                                       tag='psb')
                    for h in range(2):
                        o = a + piece * 1024 + h * 512
                        for kk in range(KK):
                            nc.tensor.matmul(ps[:, h * 512:(h + 1) * 512],
                                             xts[kk][:, a:a + 128],
                                             xts[kk][:, o:o + 512],
                                             start=(kk == 0), stop=(kk == KK - 1))
                    nc.vector.tensor_tensor(
                        w[:, piece * 1024:(piece + 1) * 1024],
                        mb[:, piece * 1024:(piece + 1) * 1024], ps[:, :],
                        op=Alu.mult)
                pt = pst_pool.tile([128, 128], f32, tag='pst')
                o = a + 2048
                for kk in range(KK):
                    nc.tensor.matmul(pt[:, :], xts[kk][:, a:a + 128],
                                     xts[kk][:, o:o + 128],
                                     start=(kk == 0), stop=(kk == KK - 1))
                nc.vector.tensor_tensor(w[:, 2048:2176], mb[:, 2048:2176],
                                        pt[:, :], op=Alu.mult)
                # Per-row top-8 candidates of w.
                nc.vector.max(outt[:, t * 8:(t + 1) * 8], w[:, :])
                # Sign-bracketed zero-loss counts on ACT.
                z1 = zs_pool.tile([128, W_RECT], f32, tag='z1')
                nc.scalar.activation(z1[:, :], w[:, :], Act.Sign,
                                     bias=bias_hi[:, :],
                                     accum_out=outt[:, 36 + t:37 + t])
                z2 = zs_pool.tile([128, W_RECT], f32, tag='z2')
                nc.scalar.activation(z2[:, :], w[:, :], Act.Sign,
                                     bias=bias_lo[:, :],
                                     accum_out=outt[:, 32 + t:33 + t])

            nc.sync.dma_start(outp_d[:, :], outt[:, :])

    nc.compile()
    return nc


def _numpy_fallback(x, t):
    """Faithful f32 numpy recompute of the full reference (safety net)."""
    sim = x @ x.T
    same = t[:, None] == t[None, :]
    eye = np.eye(N, dtype=bool)
    pos = same & ~eye
    neg = ~same
    pos_l = np.maximum(MARGIN + BETA - sim, 0.0).astype(np.float32)
    neg_l = np.maximum(MARGIN + sim - BETA, 0.0).astype(np.float32)
    valid = pos | neg
    pair = np.where(pos, pos_l, neg_l)
    zeros = int((valid & (pair == 0.0)).sum())
    masked = np.where(valid, pair, -np.inf).ravel()
    top = np.sort(masked)[-TOPK:]
    loss = np.float32(top.astype(np.float64).mean())
    mean_pos = np.float32(sim[pos].astype(np.float64).sum() / pos.sum())
    mean_neg = np.float32(sim[neg].astype(np.float64).sum() / neg.sum())
    return loss, np.int32(zeros), mean_pos, mean_neg


def kernel(**inputs):
    from concourse.bass_utils import run_bass_kernel_spmd

    x = np.ascontiguousarray(inputs['inputs'].astype(np.float32, copy=False))
    t = np.asarray(inputs['targets'])
    t_i = t.astype(np.int64)
    t16 = t.astype(np.float16)
    t32 = t.astype(np.float32)

    if 'nc' not in _prog_cache:
        _prog_cache['nc'] = _build_program()
        pj, xj = np.arange(128)[:, None], np.arange(W_RECT)[None, :]
        dj = xj - pj
        _prog_cache['jmask'] = ((dj >= 1) & (dj <= HB - 1)).astype(np.float16)
    nc = _prog_cache['nc']
    jmask = _prog_cache['jmask']

    xt = _round_f32r(np.ascontiguousarray(x.T))          # [D, N] f32r values
    xt2 = np.concatenate([xt, xt[:, :XCOLS - N]], axis=1)   # wrap for rotation
    t16w = np.concatenate([t16, t16[:XCOLS - N]])
    in_maps = []
    for c in range(NCORES):
        sh = c * R
        in_maps.append({
            'xtr': np.ascontiguousarray(
                xt2[:, sh:sh + XCOLS].reshape(KK, 128, XCOLS)),
            'tgr': np.ascontiguousarray(t16w[sh:sh + XCOLS]),
            'tgf': np.ascontiguousarray(
                t32[sh:sh + R].reshape(NT, 128).T),
            'jmask': jmask,
        })

    res = run_bass_kernel_spmd(nc, in_maps, core_ids=list(range(NCORES)))

    cands, zlos, zhis = [], [], []
    for r in res.results:
        o = r['outp']                                   # [128, 40]
        cands.append(o[:, 0:32].reshape(128, NT, 8).transpose(1, 0, 2)
                     .reshape(R, 8))
        zlos.append(o[:, 32:36].T.reshape(R))
        zhis.append(o[:, 36:40].T.reshape(R))
    cand = np.concatenate(cands, axis=0)                # [N, 8]
    zsum_lo = np.concatenate(zlos)                      # [N]
    zsum_hi = np.concatenate(zhis)

    x64 = x.astype(np.float64)
    # ---- antipodal (d = 2048) pairs: fixed index set, exact on host ----
    ai = np.arange(HB)
    s_ant = np.einsum('nd,nd->n', x64[ai], x64[ai + HB])
    w_ant = 0.5 * np.where(t_i[ai] == t_i[ai + HB], -1.0, 1.0) * s_ant

    # ---- top-10 unique pairs (x2 = reference top-20) ----
    # device candidates cover d in [1, 2047] once each; junk cells read 0.
    merged = np.concatenate([cand.ravel(), w_ant])
    top10 = np.sort(merged)[-(TOPK // 2):]
    t10 = top10[0]
    sufficiency_ok = bool((cand[:, 7] <= t10).all()) and t10 > 1e-6

    # ---- zero count guards (device counts in-band cells only) ----
    c_lo = (W_RECT - zsum_lo) / 2.0
    c_hi = (W_RECT - zsum_hi) / 2.0
    zeros_ok = (np.all(c_lo == 0.0) and np.all(c_hi == 0.0)
                and not np.any(w_ant <= ZTHR + DELTA))
    if not (sufficiency_ok and zeros_ok):
        return _numpy_fallback(x, t_i)
    num_zeros = 0

    top20 = np.repeat(top10[::-1], 2)
    loss = np.float32(np.maximum(MARGIN + 2.0 * top20.astype(np.float64), 0.0).mean())

    # ---- exact f64 stats on host ----
    G = np.zeros((int(t_i.max()) + 1, D), dtype=np.float64)
    np.add.at(G, t_i, x64)
    cls_sq = float((G * G).sum())
    diag_sq = float((x64 * x64).sum())
    cnt = np.bincount(t_i)
    pos_cnt = int((cnt.astype(np.int64) * (cnt - 1)).sum())
    neg_cnt = N * N - int((cnt.astype(np.int64) ** 2).sum())
    tot = x64.sum(axis=0)
    total_sq = float(tot @ tot)
    mean_pos = np.float32((cls_sq - diag_sq) / pos_cnt)
    mean_neg = np.float32((total_sq - cls_sq) / neg_cnt)

    return loss, np.int32(num_zeros), mean_pos, mean_neg
